# revision 17
# baseline (speedup 1.0000x reference)
"""CRF loss (nn_CRFLayer) on 8 Trainium2 NeuronCores.

Data parallel over batch (per sharding hint): B=4096 -> 8 cores x 512 seqs.
The forward recurrence runs in the exp domain with a constant per-step
shift c0 baked into the emissions:

    v_{s+1} = (M^T v_s) * exp(F_s - c0),   M[f,t] = exp(transitions[t,f])

Each step is one 128x128 bf16 matmul plus one elementwise multiply, so the
chain is latency-bound. Three exact tricks minimize the sequential depth
and the per-step cost:

1. Sink slot. Tag 31 (STOP) is structurally dead (its transitions are
   -1e4 -> exact zeros in exp domain), so M gets a ones-column into slot
   31 with unit self-loop; host-baked emissions keep the sink 0 while
   live (ef=0), capture sum_t v_len at s==len (ef=1), freeze after
   (ef=1). No masks or per-step extraction.

2. Forward-backward split:  1^T P_511..P_0 v0 =
   (P^T_256..P^T_511 1_live) . (P_255..P_0 v0), so a backward recurrence
   r_s = Mhat (e_s * r_{s+1}) over reversed slices runs CONCURRENTLY with
   the forward one -- two independent 256-step chains instead of one
   512-step chain. Variable lengths are exact via an injection slot
   (dead tag 30 in the backward matrix): ehat[30]=1 at s==len makes
   r_len = 1_live; a constant-one slot (tag 31) keeps the injection fed.
   The meet q = v_256 . r_256 automatically equals the forward sink
   capture for len<=255 and the true lse for len>=256.

3. Shared stationary weights. Chain A stacks [fwd g0; fwd g1; bwd g0;
   bwd g1] on the partition axis, chain B the same for groups 2,3, so
   every main-loop matmul uses the SAME block-diagonal lhsT
   diag(m_f, m_f, Mhat^T, Mhat^T) and the compiler's LDWEIGHTS dedup
   (--enable-ldw-opt) removes the per-step weight reloads. The meet is
   partition-aligned via a final lhsT that routes Mhat t_256 onto
   partitions 0..63 where v_256 lives.

Host: bakes shifted/masked emissions (bf16), computes the gold score
(pure index gathers + sums, like the baseline's pairval marshalling),
and assembles  loss = mean(ln(q) + c0*len - gold).
"""
import sys
import numpy as np
import ml_dtypes

sys.path.insert(0, "/opt/trn_rl_repo")

B, S, T = 4096, 512, 32
START, STOP = 30, 31
NEG = -10000.0
NCORES = 8
BC = B // NCORES          # 512 sequences per core
G = 4                     # groups per core
P = 128                   # partitions
HALF = 64                 # fwd lives on partitions 0..63, bwd on 64..127
C0 = 4.382                # constant per-step log-domain shift
DEAD = -60000.0           # exp() underflows to exactly 0
MID = S // 2              # 256 slices per direction
# graduated chunk sizes: tiny first chunk so the recurrence starts as soon
# as possible; steady-state chunks amortize DMA/exp overheads
CHUNKS = [4, 8, 16, 32, 36, 40, 40, 40, 40]
assert sum(CHUNKS) == MID

bf16 = ml_dtypes.bfloat16

_compiled = None


def _build_bass():
    import concourse.bass as bass
    import concourse.mybir as mybir
    from concourse.tile import TileContext

    f32 = mybir.dt.float32
    bf = mybir.dt.bfloat16
    AF = mybir.ActivationFunctionType

    nc = bass.Bass()
    FA_h = nc.dram_tensor("F_a", [P, MID, P], bf, kind="ExternalInput")
    FB_h = nc.dram_tensor("F_b", [P, MID, P], bf, kind="ExternalInput")
    mc_h = nc.dram_tensor("m_combo", [P, P], bf, kind="ExternalInput")
    mfin_h = nc.dram_tensor("m_fin", [P, HALF], bf, kind="ExternalInput")
    ones_h = nc.dram_tensor("ones2", [HALF, 2], bf, kind="ExternalInput")
    q_h = nc.dram_tensor("q_out", [G, P], f32, kind="ExternalOutput")

    with TileContext(nc) as tc:
        with (
            tc.tile_pool(name="singles", bufs=1) as singles,
            tc.tile_pool(name="fpool", bufs=2) as fpool,
            tc.tile_pool(name="epool", bufs=2) as epool,
            tc.tile_pool(name="state", bufs=4) as state,
            tc.tile_pool(name="small", bufs=2) as small,
            tc.tile_pool(name="ps_a", bufs=2, space="PSUM") as ps_a,
            tc.tile_pool(name="ps_b", bufs=2, space="PSUM") as ps_b,
            tc.tile_pool(name="ps_q", bufs=2, space="PSUM") as ps_q,
        ):
            mc_sb = singles.tile([P, P], bf)
            nc.sync.dma_start(out=mc_sb[:], in_=mc_h[:])
            mfin_sb = singles.tile([P, HALF], bf)
            nc.sync.dma_start(out=mfin_sb[:], in_=mfin_h[:])
            ones_sb = singles.tile([HALF, 2], bf)
            nc.sync.dma_start(out=ones_sb[:], in_=ones_h[:])

            # chain states come straight out of slice 0 of each stream
            # (host folds the v_1 / t_511 inits into those slices)
            xa = None
            xb = None
            s0 = 0
            for ch in CHUNKS:
                fac = fpool.tile([P, ch, P], bf, tag="fa")
                nc.sync.dma_start(out=fac[:], in_=FA_h[:, s0:s0 + ch, :])
                fbc = fpool.tile([P, ch, P], bf, tag="fb")
                nc.sync.dma_start(out=fbc[:], in_=FB_h[:, s0:s0 + ch, :])
                eac = epool.tile([P, ch, P], bf, tag="ea")
                nc.scalar.activation(eac[:], fac[:], AF.Exp)
                ebc = epool.tile([P, ch, P], bf, tag="eb")
                nc.scalar.activation(ebc[:], fbc[:], AF.Exp)
                for sl in range(ch):
                    if s0 + sl == 0:
                        xa = eac[:, 0, :]
                        xb = ebc[:, 0, :]
                        continue
                    psa = ps_a.tile([P, P], f32, tag="pa")
                    nc.tensor.matmul(psa[:], lhsT=mc_sb[:], rhs=xa[:],
                                     start=True, stop=True)
                    a2 = state.tile([P, P], bf, tag="a")
                    nc.vector.tensor_mul(a2[:], psa[:], eac[:, sl, :])
                    xa = a2
                    psb = ps_b.tile([P, P], f32, tag="pb")
                    nc.tensor.matmul(psb[:], lhsT=mc_sb[:], rhs=xb[:],
                                     start=True, stop=True)
                    b2 = state.tile([P, P], bf, tag="b")
                    nc.vector.tensor_mul(b2[:], psb[:], ebc[:, sl, :])
                    xb = b2
                s0 += ch

            # meet: r_256 = Mhat t_256 routed onto partitions 0..63,
            # w = r .* v_256, then per-group partition sums
            for idx, x in enumerate((xa, xb)):
                psr = ps_q.tile([HALF, P], f32, tag="pr")
                nc.tensor.matmul(psr[:], lhsT=mfin_sb[:], rhs=x[:],
                                 start=True, stop=True)
                w = state.tile([HALF, P], bf, tag="w")
                nc.vector.tensor_mul(w[:], psr[:], x[0:HALF, :])
                psq = ps_q.tile([2, P], f32, tag="pq")
                nc.tensor.matmul(psq[:], lhsT=ones_sb[:], rhs=w[:],
                                 start=True, stop=True)
                out_sb = small.tile([2, P], f32, tag="o")
                nc.scalar.copy(out_sb[:], psq[:])
                nc.sync.dma_start(out=q_h[2 * idx:2 * idx + 2, :],
                                  in_=out_sb[:])

    return nc


def _masked_streams(feats, lengths, transitions):
    """Per-core list of (Ff, Fb): masked/shifted log emission streams,
    each [G, T, MID, 128] float32 (fwd slices 0..255 / bwd 511..256)."""
    out = []
    for core in range(NCORES):
        sl = slice(core * BC, (core + 1) * BC)
        fc = feats[sl].reshape(G, P, S, T).transpose(0, 3, 2, 1)  # [G,T,S,128]
        lens = lengths[sl].reshape(G, P)
        live = np.arange(S)[None, :, None] < lens[:, None, :]     # [G,S,128]
        Fm = np.where(live[:, None, :, :], fc - C0, DEAD)         # [G,T,S,128]

        # ---- forward slices 0..MID-1 ----
        Ff = Fm[:, :, :MID, :].copy()
        Ff[:, STOP, :, :] = np.where(live[:, :MID, :], DEAD, 0.0)  # sink
        Ff[:, START, :, :] = DEAD
        # fold v_0 -> v_1 into slice 0: v_1 = exp(F_0 + trans[:,START] - c0)
        Ff[:, :, 0, :] += transitions[:, START][None, :, None]
        Ff[:, START, 0, :] = DEAD

        # ---- backward slices 511..256 (k <-> s=511-k) ----
        Fb = Fm[:, :, S - 1:MID - 1:-1, :].copy()                  # [G,T,MID,128]
        s_k = np.arange(S - 1, MID - 1, -1)                        # [MID]
        inj = s_k[None, :, None] == lens[:, None, :]               # [G,MID,128]
        Fb[:, START, :, :] = np.where(inj, 0.0, DEAD)              # injection
        Fb[:, STOP, :, :] = 0.0                                    # const-one
        # fold r_512 into k=0: live slots exist only for len==512
        Fb[:, :30, 0, :] += np.where(lens == S, 0.0, DEAD)[:, None, :]
        Fb = np.maximum(Fb, -200000.0)
        out.append((Ff, Fb))
    return out


def _host_inputs(feats, tags, lengths, transitions):
    feats = np.asarray(feats, np.float32)
    lengths = np.asarray(lengths).astype(np.int64)
    transitions = np.asarray(transitions, np.float32)

    m = np.exp(transitions.T.astype(np.float64))  # m[f,t] = exp(trans[t,f])
    m_f = m.copy()
    m_f[:, STOP] = 0.0
    m_f[:30, STOP] = 1.0   # sink ones-column
    m_f[STOP, STOP] = 1.0  # sink self-loop
    # backward Mhat: live block = m; injection col 30; const-one slot 31
    Mh = np.zeros((T, T), np.float64)
    Mh[:30, :30] = m[:30, :30]
    Mh[30, STOP] = 1.0     # keeps injector slot fed from const-one
    Mh[STOP, STOP] = 1.0   # const-one self-loop
    Mh[:30, 30] = 1.0      # injection column into live tags
    MhT = Mh.T

    # combo lhsT: diag(m_f, m_f, MhT, MhT)
    m_combo = np.zeros((P, P), np.float64)
    for i, blk in enumerate((m_f, m_f, MhT, MhT)):
        m_combo[i * T:(i + 1) * T, i * T:(i + 1) * T] = blk
    m_combo = m_combo.astype(bf16)
    # final lhsT: routes Mhat t(g) from partitions 64..127 onto 0..63
    m_fin = np.zeros((P, HALF), np.float64)
    m_fin[64:96, 0:32] = MhT
    m_fin[96:128, 32:64] = MhT
    m_fin = m_fin.astype(bf16)
    ones2 = np.zeros((HALF, 2), np.float32)
    ones2[0:32, 0] = 1.0
    ones2[32:64, 1] = 1.0
    ones2 = ones2.astype(bf16)

    streams = _masked_streams(feats, lengths, transitions)
    per_core = []
    for core in range(NCORES):
        Ff, Fb = streams[core]
        # chain A: [fwd g0; fwd g1; bwd g0; bwd g1]; chain B: groups 2,3
        FA = np.concatenate([Ff[0], Ff[1], Fb[0], Fb[1]], axis=0)  # [128,MID,128]
        FBs = np.concatenate([Ff[2], Ff[3], Fb[2], Fb[3]], axis=0)
        per_core.append({
            "F_a": np.ascontiguousarray(FA.astype(bf16)),
            "F_b": np.ascontiguousarray(FBs.astype(bf16)),
            "m_combo": m_combo,
            "m_fin": m_fin,
            "ones2": ones2,
        })
    return per_core


def _gold(feats, tags, lengths, transitions):
    """Exact gold score minus its -1e4 STOP term (cancels forward's)."""
    feats = np.asarray(feats, np.float64)
    tags = np.asarray(tags).astype(np.int64)
    lengths = np.asarray(lengths).astype(np.int64)
    trans = np.asarray(transitions, np.float64)

    tags_prev = np.concatenate(
        [np.full((B, 1), START, np.int64), tags[:, :-1]], axis=1)
    pairval = trans[tags, tags_prev]                      # [B, S]
    smask = np.arange(S)[None, :] < lengths[:, None]
    trans_score = np.where(smask, pairval, 0.0).sum(axis=1)
    emit_vals = np.take_along_axis(feats, tags[:, :, None], axis=2)[:, :, 0]
    emit_score = np.where(smask, emit_vals, 0.0).sum(axis=1)
    return trans_score + emit_score


def kernel(feats, tags, lengths, transitions):
    global _compiled
    from concourse.bass_utils import run_bass_kernel_spmd
    import waitfix_embedded  # noqa: F401  (installs on import)

    if _compiled is None:
        _compiled = _build_bass()
    nc = _compiled
    in_maps = _host_inputs(feats, tags, lengths, transitions)
    res = run_bass_kernel_spmd(nc, in_maps, core_ids=list(range(NCORES)))

    lengths64 = np.asarray(lengths).astype(np.int64)
    gold = _gold(feats, tags, lengths, transitions)
    fwd = np.empty(B, np.float64)
    for core, r in enumerate(res.results):
        q = r["q_out"].astype(np.float64)                 # [G, 128]
        sl = slice(core * BC, (core + 1) * BC)
        fwd[sl] = np.log(q).reshape(BC) + C0 * lengths64[sl]
    return np.float32(np.mean(fwd - gold))


# ---- embedded waitfix module (kernel.py must be self-contained) ----
import types as _types  # noqa: E402

_wf_src = '''
import json

MAX_WAITS = 1

def dedupe_ldweights(bir):
    """Drop Ldweights that reload the weights already resident in the PE
    array (same stationary AP as the previous load, no intervening
    self-loading matmul). Their sync_info is merged into the next kept
    instruction so no dependency edges are lost."""
    def sig_of(inst):
        return json.dumps(
            [inst.get("ins"), inst.get("perf_mode"), inst.get("is_transpose"),
             inst.get("tile_position")], sort_keys=True)

    n_drop = 0
    for fn in bir["functions"]:
        for blk in fn["blocks"]:
            out = []
            last_sig = None
            pend_w, pend_u = [], []
            for inst in blk["instructions"]:
                op = inst.get("opcode")
                if op == "Ldweights":
                    sig = sig_of(inst)
                    if sig == last_sig:
                        si = inst.get("sync_info") or {}
                        pend_w += si.get("on_wait") or []
                        pend_u += si.get("on_update") or []
                        n_drop += 1
                        continue
                    last_sig = sig
                elif op in ("Matmult", "MatmultMx"):
                    if inst.get("is_transpose") or inst.get("ldweights"):
                        last_sig = None
                if pend_w or pend_u:
                    si = inst.get("sync_info")
                    if si is None:
                        si = {"on_update": [], "on_wait": []}
                        inst["sync_info"] = si
                    si["on_wait"] = (si.get("on_wait") or []) + pend_w
                    si["on_update"] = (si.get("on_update") or []) + pend_u
                    pend_w, pend_u = [], []
                out.append(inst)
            assert not (pend_w or pend_u), "dangling ldweights sync"
            blk["instructions"] = out
    return bir

def split_sync_waits(bir_bytes, max_waits=MAX_WAITS):
    bir = dedupe_ldweights(json.loads(bir_bytes))
    n_split = 0
    for fn in bir["functions"]:
        for blk in fn["blocks"]:
            out = []
            for inst in blk["instructions"]:
                si = inst.get("sync_info")
                waits = (si or {}).get("on_wait") or []
                if len(waits) > max_waits:
                    k = 0
                    while len(waits) > max_waits:
                        chunk, waits = waits[:max_waits], waits[max_waits:]
                        out.append({
                            "debug": inst.get("debug", 0),
                            "engine": inst["engine"],
                            "ins": [], "is_reset_sema": False,
                            "name": inst["name"] + "-wsplit%d" % k,
                            "opcode": "NoOp", "outs": [],
                            "sync_info": {"on_update": [], "on_wait": chunk},
                        })
                        k += 1
                    si["on_wait"] = waits
                    n_split += 1
                out.append(inst)
            blk["instructions"] = out
    return json.dumps(bir).encode()

def install():
    import concourse.bass2jax as bass2jax
    if getattr(bass2jax, "_waitfix_installed", False):
        return
    orig = bass2jax.compile_bir_kernel
    def patched(bir_json, tmpdir, neff_name="file.neff"):
        return orig(split_sync_waits(bir_json), tmpdir, neff_name)
    bass2jax.compile_bir_kernel = patched
    bass2jax._waitfix_installed = True

install()
'''
if "waitfix_embedded" not in sys.modules:
    _mod = _types.ModuleType("waitfix_embedded")
    exec(_wf_src, _mod.__dict__)
    sys.modules["waitfix_embedded"] = _mod


if __name__ == "__main__":
    import refcache
    inputs, exp = refcache.load()
    out = kernel(**inputs)
    rel = abs(float(out) - float(exp)) / max(abs(float(exp)), 1e-9)
    print("kernel:", out, "expected:", exp, "rel err:", rel)


# revision 19
# speedup vs baseline: 1.3127x; 1.3127x over previous
"""CRF loss (nn_CRFLayer) on 8 Trainium2 NeuronCores.

Data parallel over batch (per sharding hint): B=4096 -> 8 cores x 512 seqs.
The forward recurrence runs in the exp domain with a constant per-step
shift c0 baked into the emissions:

    v_{s+1} = (M^T v_s) * exp(F_s - c0),   M[f,t] = exp(transitions[t,f])

Each step is one 128x128 bf16 matmul plus one elementwise multiply, so the
chain is latency-bound. Three exact tricks minimize the sequential depth
and the per-step cost:

1. Sink slot. Tag 31 (STOP) is structurally dead (its transitions are
   -1e4 -> exact zeros in exp domain), so M gets a ones-column into slot
   31 with unit self-loop; host-baked emissions keep the sink 0 while
   live (ef=0), capture sum_t v_len at s==len (ef=1), freeze after
   (ef=1). No masks or per-step extraction.

2. Forward-backward split:  1^T P_511..P_0 v0 =
   (P^T_256..P^T_511 1_live) . (P_255..P_0 v0), so a backward recurrence
   r_s = Mhat (e_s * r_{s+1}) over reversed slices runs CONCURRENTLY with
   the forward one -- two independent 256-step chains instead of one
   512-step chain. Variable lengths are exact via an injection slot
   (dead tag 30 in the backward matrix): ehat[30]=1 at s==len makes
   r_len = 1_live; a constant-one slot (tag 31) keeps the injection fed.
   The meet q = v_256 . r_256 automatically equals the forward sink
   capture for len<=255 and the true lse for len>=256.

3. Shared stationary weights. Chain A stacks [fwd g0; fwd g1; bwd g0;
   bwd g1] on the partition axis, chain B the same for groups 2,3, so
   every main-loop matmul uses the SAME block-diagonal lhsT
   diag(m_f, m_f, Mhat^T, Mhat^T) and the compiler's LDWEIGHTS dedup
   (--enable-ldw-opt) removes the per-step weight reloads. The meet is
   partition-aligned via a final lhsT that routes Mhat t_256 onto
   partitions 0..63 where v_256 lives.

Host: bakes shifted/masked emissions (bf16), computes the gold score
(pure index gathers + sums, like the baseline's pairval marshalling),
and assembles  loss = mean(ln(q) + c0*len - gold).
"""
import sys
import numpy as np
import ml_dtypes

sys.path.insert(0, "/opt/trn_rl_repo")

B, S, T = 4096, 512, 32
START, STOP = 30, 31
NEG = -10000.0
NCORES = 8
BC = B // NCORES          # 512 sequences per core
G = 4                     # groups per core
P = 128                   # partitions
HALF = 64                 # fwd lives on partitions 0..63, bwd on 64..127
C0 = 4.382                # constant per-step log-domain shift
DEAD = -60000.0           # exp() underflows to exactly 0
MID = S // 2              # 256 slices per direction
# graduated chunk sizes: tiny first chunk so the recurrence starts as soon
# as possible; steady-state chunks amortize DMA/exp overheads
CHUNKS = [4, 28, 32, 32, 32, 32, 32, 32, 32]
assert sum(CHUNKS) == MID

bf16 = ml_dtypes.bfloat16

_compiled = None


def _build_bass():
    import concourse.bass as bass
    import concourse.mybir as mybir
    from concourse.tile import TileContext

    f32 = mybir.dt.float32
    bf = mybir.dt.bfloat16
    AF = mybir.ActivationFunctionType

    nc = bass.Bass()
    FA_h = nc.dram_tensor("F_a", [P, MID, P], bf, kind="ExternalInput")
    FB_h = nc.dram_tensor("F_b", [P, MID, P], bf, kind="ExternalInput")
    mc_h = nc.dram_tensor("m_combo", [P, P], bf, kind="ExternalInput")
    mfin_h = nc.dram_tensor("m_fin", [P, HALF], bf, kind="ExternalInput")
    ones_h = nc.dram_tensor("ones2", [HALF, 2], bf, kind="ExternalInput")
    q_h = nc.dram_tensor("q_out", [G, P], f32, kind="ExternalOutput")

    with TileContext(nc) as tc:
        with (
            tc.tile_pool(name="singles", bufs=1) as singles,
            tc.tile_pool(name="fpool", bufs=2) as fpool,
            tc.tile_pool(name="epool", bufs=2) as epool,
            tc.tile_pool(name="state", bufs=4) as state,
            tc.tile_pool(name="small", bufs=2) as small,
            tc.tile_pool(name="ps_a", bufs=2, space="PSUM") as ps_a,
            tc.tile_pool(name="ps_b", bufs=2, space="PSUM") as ps_b,
            tc.tile_pool(name="ps_q", bufs=2, space="PSUM") as ps_q,
        ):
            mc_sb = singles.tile([P, P], bf)
            nc.sync.dma_start(out=mc_sb[:], in_=mc_h[:])
            mfin_sb = singles.tile([P, HALF], bf)
            nc.sync.dma_start(out=mfin_sb[:], in_=mfin_h[:])
            ones_sb = singles.tile([HALF, 2], bf)
            nc.sync.dma_start(out=ones_sb[:], in_=ones_h[:])

            # chain states come straight out of slice 0 of each stream
            # (host folds the v_1 / t_511 inits into those slices)
            xa = None
            xb = None
            s0 = 0
            for ch in CHUNKS:
                fac = fpool.tile([P, ch, P], bf, tag="fa")
                nc.sync.dma_start(out=fac[:], in_=FA_h[:, s0:s0 + ch, :])
                fbc = fpool.tile([P, ch, P], bf, tag="fb")
                nc.sync.dma_start(out=fbc[:], in_=FB_h[:, s0:s0 + ch, :])
                eac = epool.tile([P, ch, P], bf, tag="ea")
                nc.scalar.activation(eac[:], fac[:], AF.Exp)
                ebc = epool.tile([P, ch, P], bf, tag="eb")
                nc.scalar.activation(ebc[:], fbc[:], AF.Exp)
                for sl in range(ch):
                    if s0 + sl == 0:
                        xa = eac[:, 0, :]
                        xb = ebc[:, 0, :]
                        continue
                    psa = ps_a.tile([P, P], f32, tag="pa")
                    nc.tensor.matmul(psa[:], lhsT=mc_sb[:], rhs=xa[:],
                                     start=True, stop=True)
                    a2 = state.tile([P, P], bf, tag="a")
                    nc.vector.tensor_mul(a2[:], psa[:], eac[:, sl, :])
                    xa = a2
                    psb = ps_b.tile([P, P], f32, tag="pb")
                    nc.tensor.matmul(psb[:], lhsT=mc_sb[:], rhs=xb[:],
                                     start=True, stop=True)
                    b2 = state.tile([P, P], bf, tag="b")
                    nc.vector.tensor_mul(b2[:], psb[:], ebc[:, sl, :])
                    xb = b2
                s0 += ch

            # meet: r_256 = Mhat t_256 routed onto partitions 0..63,
            # w = r .* v_256, then per-group partition sums
            for idx, x in enumerate((xa, xb)):
                psr = ps_q.tile([HALF, P], f32, tag="pr")
                nc.tensor.matmul(psr[:], lhsT=mfin_sb[:], rhs=x[:],
                                 start=True, stop=True)
                w = state.tile([HALF, P], bf, tag="w")
                nc.vector.tensor_mul(w[:], psr[:], x[0:HALF, :])
                psq = ps_q.tile([2, P], f32, tag="pq")
                nc.tensor.matmul(psq[:], lhsT=ones_sb[:], rhs=w[:],
                                 start=True, stop=True)
                out_sb = small.tile([2, P], f32, tag="o")
                nc.scalar.copy(out_sb[:], psq[:])
                nc.sync.dma_start(out=q_h[2 * idx:2 * idx + 2, :],
                                  in_=out_sb[:])

    return nc


def _masked_streams(feats, lengths, transitions):
    """Per-core list of (Ff, Fb): masked/shifted log emission streams,
    each [G, T, MID, 128] float32 (fwd slices 0..255 / bwd 511..256)."""
    out = []
    for core in range(NCORES):
        sl = slice(core * BC, (core + 1) * BC)
        fc = feats[sl].reshape(G, P, S, T).transpose(0, 3, 2, 1)  # [G,T,S,128]
        lens = lengths[sl].reshape(G, P)
        live = np.arange(S)[None, :, None] < lens[:, None, :]     # [G,S,128]
        Fm = np.where(live[:, None, :, :], fc - C0, DEAD)         # [G,T,S,128]

        # ---- forward slices 0..MID-1 ----
        Ff = Fm[:, :, :MID, :].copy()
        Ff[:, STOP, :, :] = np.where(live[:, :MID, :], DEAD, 0.0)  # sink
        Ff[:, START, :, :] = DEAD
        # fold v_0 -> v_1 into slice 0: v_1 = exp(F_0 + trans[:,START] - c0)
        Ff[:, :, 0, :] += transitions[:, START][None, :, None]
        Ff[:, START, 0, :] = DEAD

        # ---- backward slices 511..256 (k <-> s=511-k) ----
        Fb = Fm[:, :, S - 1:MID - 1:-1, :].copy()                  # [G,T,MID,128]
        s_k = np.arange(S - 1, MID - 1, -1)                        # [MID]
        inj = s_k[None, :, None] == lens[:, None, :]               # [G,MID,128]
        Fb[:, START, :, :] = np.where(inj, 0.0, DEAD)              # injection
        Fb[:, STOP, :, :] = 0.0                                    # const-one
        # fold r_512 into k=0: live slots exist only for len==512
        Fb[:, :30, 0, :] += np.where(lens == S, 0.0, DEAD)[:, None, :]
        Fb = np.maximum(Fb, -200000.0)
        out.append((Ff, Fb))
    return out


def _host_inputs(feats, tags, lengths, transitions):
    feats = np.asarray(feats, np.float32)
    lengths = np.asarray(lengths).astype(np.int64)
    transitions = np.asarray(transitions, np.float32)

    m = np.exp(transitions.T.astype(np.float64))  # m[f,t] = exp(trans[t,f])
    m_f = m.copy()
    m_f[:, STOP] = 0.0
    m_f[:30, STOP] = 1.0   # sink ones-column
    m_f[STOP, STOP] = 1.0  # sink self-loop
    # backward Mhat: live block = m; injection col 30; const-one slot 31
    Mh = np.zeros((T, T), np.float64)
    Mh[:30, :30] = m[:30, :30]
    Mh[30, STOP] = 1.0     # keeps injector slot fed from const-one
    Mh[STOP, STOP] = 1.0   # const-one self-loop
    Mh[:30, 30] = 1.0      # injection column into live tags
    MhT = Mh.T

    # combo lhsT: diag(m_f, m_f, MhT, MhT)
    m_combo = np.zeros((P, P), np.float64)
    for i, blk in enumerate((m_f, m_f, MhT, MhT)):
        m_combo[i * T:(i + 1) * T, i * T:(i + 1) * T] = blk
    m_combo = m_combo.astype(bf16)
    # final lhsT: routes Mhat t(g) from partitions 64..127 onto 0..63
    m_fin = np.zeros((P, HALF), np.float64)
    m_fin[64:96, 0:32] = MhT
    m_fin[96:128, 32:64] = MhT
    m_fin = m_fin.astype(bf16)
    ones2 = np.zeros((HALF, 2), np.float32)
    ones2[0:32, 0] = 1.0
    ones2[32:64, 1] = 1.0
    ones2 = ones2.astype(bf16)

    streams = _masked_streams(feats, lengths, transitions)
    per_core = []
    for core in range(NCORES):
        Ff, Fb = streams[core]
        # chain A: [fwd g0; fwd g1; bwd g0; bwd g1]; chain B: groups 2,3
        FA = np.concatenate([Ff[0], Ff[1], Fb[0], Fb[1]], axis=0)  # [128,MID,128]
        FBs = np.concatenate([Ff[2], Ff[3], Fb[2], Fb[3]], axis=0)
        per_core.append({
            "F_a": np.ascontiguousarray(FA.astype(bf16)),
            "F_b": np.ascontiguousarray(FBs.astype(bf16)),
            "m_combo": m_combo,
            "m_fin": m_fin,
            "ones2": ones2,
        })
    return per_core


def _gold(feats, tags, lengths, transitions):
    """Exact gold score minus its -1e4 STOP term (cancels forward's)."""
    feats = np.asarray(feats, np.float64)
    tags = np.asarray(tags).astype(np.int64)
    lengths = np.asarray(lengths).astype(np.int64)
    trans = np.asarray(transitions, np.float64)

    tags_prev = np.concatenate(
        [np.full((B, 1), START, np.int64), tags[:, :-1]], axis=1)
    pairval = trans[tags, tags_prev]                      # [B, S]
    smask = np.arange(S)[None, :] < lengths[:, None]
    trans_score = np.where(smask, pairval, 0.0).sum(axis=1)
    emit_vals = np.take_along_axis(feats, tags[:, :, None], axis=2)[:, :, 0]
    emit_score = np.where(smask, emit_vals, 0.0).sum(axis=1)
    return trans_score + emit_score


def kernel(feats, tags, lengths, transitions):
    global _compiled
    from concourse.bass_utils import run_bass_kernel_spmd
    import waitfix_embedded  # noqa: F401  (installs on import)

    if _compiled is None:
        _compiled = _build_bass()
    nc = _compiled
    in_maps = _host_inputs(feats, tags, lengths, transitions)
    res = run_bass_kernel_spmd(nc, in_maps, core_ids=list(range(NCORES)))

    lengths64 = np.asarray(lengths).astype(np.int64)
    gold = _gold(feats, tags, lengths, transitions)
    fwd = np.empty(B, np.float64)
    for core, r in enumerate(res.results):
        q = r["q_out"].astype(np.float64)                 # [G, 128]
        sl = slice(core * BC, (core + 1) * BC)
        fwd[sl] = np.log(q).reshape(BC) + C0 * lengths64[sl]
    return np.float32(np.mean(fwd - gold))


# ---- embedded waitfix module (kernel.py must be self-contained) ----
import types as _types  # noqa: E402

_wf_src = '''
import json

MAX_WAITS = 1

def dedupe_ldweights(bir):
    """Drop Ldweights that reload the weights already resident in the PE
    array (same stationary AP as the previous load, no intervening
    self-loading matmul). Their sync_info is merged into the next kept
    instruction so no dependency edges are lost."""
    def sig_of(inst):
        return json.dumps(
            [inst.get("ins"), inst.get("perf_mode"), inst.get("is_transpose"),
             inst.get("tile_position")], sort_keys=True)

    n_drop = 0
    for fn in bir["functions"]:
        for blk in fn["blocks"]:
            out = []
            last_sig = None
            pend_w, pend_u = [], []
            for inst in blk["instructions"]:
                op = inst.get("opcode")
                if op == "Ldweights":
                    sig = sig_of(inst)
                    if sig == last_sig:
                        si = inst.get("sync_info") or {}
                        pend_w += si.get("on_wait") or []
                        pend_u += si.get("on_update") or []
                        n_drop += 1
                        continue
                    last_sig = sig
                elif op in ("Matmult", "MatmultMx"):
                    if inst.get("is_transpose") or inst.get("ldweights"):
                        last_sig = None
                if pend_w or pend_u:
                    si = inst.get("sync_info")
                    if si is None:
                        si = {"on_update": [], "on_wait": []}
                        inst["sync_info"] = si
                    si["on_wait"] = (si.get("on_wait") or []) + pend_w
                    si["on_update"] = (si.get("on_update") or []) + pend_u
                    pend_w, pend_u = [], []
                out.append(inst)
            assert not (pend_w or pend_u), "dangling ldweights sync"
            blk["instructions"] = out
    return bir

def drop_satisfied_waits(bir):
    """Drop waits that are provably satisfied at issue: a wait on semaphore X
    by an instruction on engine E, where earlier instructions on E in the
    same block have already pushed X past the wait value (in-order queue).
    Semaphores touched by any non-inc update mode are left alone."""
    n_drop = 0
    for fn in bir["functions"]:
        for blk in fn["blocks"]:
            upd = {}      # (engine, sem_id) -> cumulative inc by that engine
            tainted = set()
            for inst in blk["instructions"]:
                e = inst.get("engine")
                si = inst.get("sync_info")
                if si and si.get("on_wait"):
                    keep = []
                    for w in si["on_wait"]:
                        if (w.get("sync_type") == "semaphore"
                                and w.get("wait_mode") == "sem-ge-imm"
                                and w["id"] not in tainted
                                and upd.get((e, w["id"]), 0) >= w["wait_value"]):
                            n_drop += 1
                            continue
                        keep.append(w)
                    si["on_wait"] = keep
                if inst.get("is_reset_sema"):
                    for u in (si or {}).get("on_update") or []:
                        tainted.add(u.get("id"))
                for u in (si or {}).get("on_update") or []:
                    if u.get("sync_type") != "semaphore":
                        continue
                    if u.get("update_mode") == "sem-inc":
                        k = (e, u["id"])
                        upd[k] = upd.get(k, 0) + u.get("update_value", 1)
                    else:
                        tainted.add(u.get("id"))
    return bir

def split_sync_waits(bir_bytes, max_waits=MAX_WAITS):
    bir = drop_satisfied_waits(dedupe_ldweights(json.loads(bir_bytes)))
    n_split = 0
    for fn in bir["functions"]:
        for blk in fn["blocks"]:
            out = []
            for inst in blk["instructions"]:
                si = inst.get("sync_info")
                waits = (si or {}).get("on_wait") or []
                if len(waits) > max_waits:
                    k = 0
                    while len(waits) > max_waits:
                        chunk, waits = waits[:max_waits], waits[max_waits:]
                        out.append({
                            "debug": inst.get("debug", 0),
                            "engine": inst["engine"],
                            "ins": [], "is_reset_sema": False,
                            "name": inst["name"] + "-wsplit%d" % k,
                            "opcode": "NoOp", "outs": [],
                            "sync_info": {"on_update": [], "on_wait": chunk},
                        })
                        k += 1
                    si["on_wait"] = waits
                    n_split += 1
                out.append(inst)
            blk["instructions"] = out
    return json.dumps(bir).encode()

def install():
    import concourse.bass2jax as bass2jax
    if getattr(bass2jax, "_waitfix_installed", False):
        return
    orig = bass2jax.compile_bir_kernel
    def patched(bir_json, tmpdir, neff_name="file.neff"):
        return orig(split_sync_waits(bir_json), tmpdir, neff_name)
    bass2jax.compile_bir_kernel = patched
    bass2jax._waitfix_installed = True

install()
'''
if "waitfix_embedded" not in sys.modules:
    _mod = _types.ModuleType("waitfix_embedded")
    exec(_wf_src, _mod.__dict__)
    sys.modules["waitfix_embedded"] = _mod


if __name__ == "__main__":
    import refcache
    inputs, exp = refcache.load()
    out = kernel(**inputs)
    rel = abs(float(out) - float(exp)) / max(abs(float(exp)), 1e-9)
    print("kernel:", out, "expected:", exp, "rel err:", rel)


# revision 22
# speedup vs baseline: 1.3653x; 1.0401x over previous
"""CRF loss (nn_CRFLayer) on 8 Trainium2 NeuronCores.

Data parallel over batch (per sharding hint): B=4096 -> 8 cores x 512 seqs.
The forward recurrence runs in the exp domain with a constant per-step
shift c0 baked into the emissions:

    v_{s+1} = (M^T v_s) * exp(F_s - c0),   M[f,t] = exp(transitions[t,f])

Each step is one 128x128 bf16 matmul plus one elementwise multiply, so the
chain is latency-bound. Three exact tricks minimize the sequential depth
and the per-step cost:

1. Sink slot. Tag 31 (STOP) is structurally dead (its transitions are
   -1e4 -> exact zeros in exp domain), so M gets a ones-column into slot
   31 with unit self-loop; host-baked emissions keep the sink 0 while
   live (ef=0), capture sum_t v_len at s==len (ef=1), freeze after
   (ef=1). No masks or per-step extraction.

2. Forward-backward split:  1^T P_511..P_0 v0 =
   (P^T_256..P^T_511 1_live) . (P_255..P_0 v0), so a backward recurrence
   r_s = Mhat (e_s * r_{s+1}) over reversed slices runs CONCURRENTLY with
   the forward one -- two independent 256-step chains instead of one
   512-step chain. Variable lengths are exact via an injection slot
   (dead tag 30 in the backward matrix): ehat[30]=1 at s==len makes
   r_len = 1_live; a constant-one slot (tag 31) keeps the injection fed.
   The meet q = v_256 . r_256 automatically equals the forward sink
   capture for len<=255 and the true lse for len>=256.

3. Shared stationary weights. Chain A stacks [fwd g0; fwd g1; bwd g0;
   bwd g1] on the partition axis, chain B the same for groups 2,3, so
   every main-loop matmul uses the SAME block-diagonal lhsT
   diag(m_f, m_f, Mhat^T, Mhat^T) and the compiler's LDWEIGHTS dedup
   (--enable-ldw-opt) removes the per-step weight reloads. The meet is
   partition-aligned via a final lhsT that routes Mhat t_256 onto
   partitions 0..63 where v_256 lives.

Host: bakes shifted/masked emissions (bf16), computes the gold score
(pure index gathers + sums, like the baseline's pairval marshalling),
and assembles  loss = mean(ln(q) + c0*len - gold).
"""
import sys
import numpy as np
import ml_dtypes

sys.path.insert(0, "/opt/trn_rl_repo")

B, S, T = 4096, 512, 32
START, STOP = 30, 31
NEG = -10000.0
NCORES = 8
BC = B // NCORES          # 512 sequences per core
G = 4                     # groups per core
P = 128                   # partitions
HALF = 64                 # fwd lives on partitions 0..63, bwd on 64..127
C0 = 4.382                # constant per-step log-domain shift
DEAD = -60000.0           # exp() underflows to exactly 0
MID = S // 2              # 256 slices per direction
# graduated chunk sizes: tiny first chunk so the recurrence starts as soon
# as possible; steady-state chunks amortize DMA/exp overheads
CHUNKS = [4, 8, 16, 28, 32, 32, 32, 32, 32, 32, 8]
assert sum(CHUNKS) == MID

bf16 = ml_dtypes.bfloat16

_compiled = None


def _build_bass():
    import concourse.bass as bass
    import concourse.mybir as mybir
    from concourse.tile import TileContext

    f32 = mybir.dt.float32
    bf = mybir.dt.bfloat16
    AF = mybir.ActivationFunctionType

    nc = bass.Bass()
    FA_h = nc.dram_tensor("F_a", [P, MID, P], bf, kind="ExternalInput")
    FB_h = nc.dram_tensor("F_b", [P, MID, P], bf, kind="ExternalInput")
    mc_h = nc.dram_tensor("m_combo", [P, P], bf, kind="ExternalInput")
    mfin_h = nc.dram_tensor("m_fin", [P, HALF], bf, kind="ExternalInput")
    ones_h = nc.dram_tensor("ones2", [HALF, 2], bf, kind="ExternalInput")
    q_h = nc.dram_tensor("q_out", [G, P], f32, kind="ExternalOutput")

    with TileContext(nc) as tc:
        with (
            tc.tile_pool(name="singles", bufs=1) as singles,
            tc.tile_pool(name="fpool", bufs=3) as fpool,
            tc.tile_pool(name="epool", bufs=3) as epool,
            tc.tile_pool(name="state", bufs=4) as state,
            tc.tile_pool(name="small", bufs=2) as small,
            tc.tile_pool(name="ps_a", bufs=2, space="PSUM") as ps_a,
            tc.tile_pool(name="ps_b", bufs=2, space="PSUM") as ps_b,
            tc.tile_pool(name="ps_q", bufs=2, space="PSUM") as ps_q,
        ):
            # chain states come straight out of slice 0 of each stream
            # (host folds the v_1 / t_511 inits into those slices).
            # Chunk-0 DMAs are issued before the constant tiles so the
            # recurrence starts as early as possible.
            mc_sb = mfin_sb = ones_sb = None
            xa = None
            xb = None
            s0 = 0
            for ci, ch in enumerate(CHUNKS):
                fac = fpool.tile([P, ch, P], bf, tag="fa")
                nc.sync.dma_start(out=fac[:], in_=FA_h[:, s0:s0 + ch, :])
                fbc = fpool.tile([P, ch, P], bf, tag="fb")
                nc.sync.dma_start(out=fbc[:], in_=FB_h[:, s0:s0 + ch, :])
                eac = epool.tile([P, ch, P], bf, tag="ea")
                nc.scalar.activation(eac[:], fac[:], AF.Exp)
                ebc = epool.tile([P, ch, P], bf, tag="eb")
                nc.scalar.activation(ebc[:], fbc[:], AF.Exp)
                if ci == 0:
                    mc_sb = singles.tile([P, P], bf)
                    nc.sync.dma_start(out=mc_sb[:], in_=mc_h[:])
                elif ci == 1:
                    mfin_sb = singles.tile([P, HALF], bf)
                    nc.sync.dma_start(out=mfin_sb[:], in_=mfin_h[:])
                    ones_sb = singles.tile([HALF, 2], bf)
                    nc.sync.dma_start(out=ones_sb[:], in_=ones_h[:])
                for sl in range(ch):
                    if s0 + sl == 0:
                        xa = eac[:, 0, :]
                        xb = ebc[:, 0, :]
                        continue
                    psa = ps_a.tile([P, P], f32, tag="pa")
                    nc.tensor.matmul(psa[:], lhsT=mc_sb[:], rhs=xa[:],
                                     start=True, stop=True)
                    a2 = state.tile([P, P], bf, tag="a")
                    nc.vector.tensor_mul(a2[:], psa[:], eac[:, sl, :])
                    xa = a2
                    psb = ps_b.tile([P, P], f32, tag="pb")
                    nc.tensor.matmul(psb[:], lhsT=mc_sb[:], rhs=xb[:],
                                     start=True, stop=True)
                    b2 = state.tile([P, P], bf, tag="b")
                    nc.vector.tensor_mul(b2[:], psb[:], ebc[:, sl, :])
                    xb = b2
                s0 += ch

            # meet: r_256 = Mhat t_256 routed onto partitions 0..63,
            # w = r .* v_256, then per-group partition sums
            for idx, x in enumerate((xa, xb)):
                psr = ps_q.tile([HALF, P], f32, tag="pr")
                nc.tensor.matmul(psr[:], lhsT=mfin_sb[:], rhs=x[:],
                                 start=True, stop=True)
                w = state.tile([HALF, P], bf, tag="w")
                nc.vector.tensor_mul(w[:], psr[:], x[0:HALF, :])
                psq = ps_q.tile([2, P], f32, tag="pq")
                nc.tensor.matmul(psq[:], lhsT=ones_sb[:], rhs=w[:],
                                 start=True, stop=True)
                out_sb = small.tile([2, P], f32, tag="o")
                nc.scalar.copy(out_sb[:], psq[:])
                nc.sync.dma_start(out=q_h[2 * idx:2 * idx + 2, :],
                                  in_=out_sb[:])

    return nc


def _masked_streams(feats, lengths, transitions):
    """Per-core list of (Ff, Fb): masked/shifted log emission streams,
    each [G, T, MID, 128] float32 (fwd slices 0..255 / bwd 511..256)."""
    out = []
    for core in range(NCORES):
        sl = slice(core * BC, (core + 1) * BC)
        fc = feats[sl].reshape(G, P, S, T).transpose(0, 3, 2, 1)  # [G,T,S,128]
        lens = lengths[sl].reshape(G, P)
        live = np.arange(S)[None, :, None] < lens[:, None, :]     # [G,S,128]
        Fm = np.where(live[:, None, :, :], fc - C0, DEAD)         # [G,T,S,128]

        # ---- forward slices 0..MID-1 ----
        Ff = Fm[:, :, :MID, :].copy()
        Ff[:, STOP, :, :] = np.where(live[:, :MID, :], DEAD, 0.0)  # sink
        Ff[:, START, :, :] = DEAD
        # fold v_0 -> v_1 into slice 0: v_1 = exp(F_0 + trans[:,START] - c0)
        Ff[:, :, 0, :] += transitions[:, START][None, :, None]
        Ff[:, START, 0, :] = DEAD

        # ---- backward slices 511..256 (k <-> s=511-k) ----
        Fb = Fm[:, :, S - 1:MID - 1:-1, :].copy()                  # [G,T,MID,128]
        s_k = np.arange(S - 1, MID - 1, -1)                        # [MID]
        inj = s_k[None, :, None] == lens[:, None, :]               # [G,MID,128]
        Fb[:, START, :, :] = np.where(inj, 0.0, DEAD)              # injection
        Fb[:, STOP, :, :] = 0.0                                    # const-one
        # fold r_512 into k=0: live slots exist only for len==512
        Fb[:, :30, 0, :] += np.where(lens == S, 0.0, DEAD)[:, None, :]
        Fb = np.maximum(Fb, -200000.0)
        out.append((Ff, Fb))
    return out


def _host_inputs(feats, tags, lengths, transitions):
    feats = np.asarray(feats, np.float32)
    lengths = np.asarray(lengths).astype(np.int64)
    transitions = np.asarray(transitions, np.float32)

    m = np.exp(transitions.T.astype(np.float64))  # m[f,t] = exp(trans[t,f])
    m_f = m.copy()
    m_f[:, STOP] = 0.0
    m_f[:30, STOP] = 1.0   # sink ones-column
    m_f[STOP, STOP] = 1.0  # sink self-loop
    # backward Mhat: live block = m; injection col 30; const-one slot 31
    Mh = np.zeros((T, T), np.float64)
    Mh[:30, :30] = m[:30, :30]
    Mh[30, STOP] = 1.0     # keeps injector slot fed from const-one
    Mh[STOP, STOP] = 1.0   # const-one self-loop
    Mh[:30, 30] = 1.0      # injection column into live tags
    MhT = Mh.T

    # combo lhsT: diag(m_f, m_f, MhT, MhT)
    m_combo = np.zeros((P, P), np.float64)
    for i, blk in enumerate((m_f, m_f, MhT, MhT)):
        m_combo[i * T:(i + 1) * T, i * T:(i + 1) * T] = blk
    m_combo = m_combo.astype(bf16)
    # final lhsT: routes Mhat t(g) from partitions 64..127 onto 0..63
    m_fin = np.zeros((P, HALF), np.float64)
    m_fin[64:96, 0:32] = MhT
    m_fin[96:128, 32:64] = MhT
    m_fin = m_fin.astype(bf16)
    ones2 = np.zeros((HALF, 2), np.float32)
    ones2[0:32, 0] = 1.0
    ones2[32:64, 1] = 1.0
    ones2 = ones2.astype(bf16)

    streams = _masked_streams(feats, lengths, transitions)
    per_core = []
    for core in range(NCORES):
        Ff, Fb = streams[core]
        # chain A: [fwd g0; fwd g1; bwd g0; bwd g1]; chain B: groups 2,3
        FA = np.concatenate([Ff[0], Ff[1], Fb[0], Fb[1]], axis=0)  # [128,MID,128]
        FBs = np.concatenate([Ff[2], Ff[3], Fb[2], Fb[3]], axis=0)
        per_core.append({
            "F_a": np.ascontiguousarray(FA.astype(bf16)),
            "F_b": np.ascontiguousarray(FBs.astype(bf16)),
            "m_combo": m_combo,
            "m_fin": m_fin,
            "ones2": ones2,
        })
    return per_core


def _gold(feats, tags, lengths, transitions):
    """Exact gold score minus its -1e4 STOP term (cancels forward's)."""
    feats = np.asarray(feats, np.float64)
    tags = np.asarray(tags).astype(np.int64)
    lengths = np.asarray(lengths).astype(np.int64)
    trans = np.asarray(transitions, np.float64)

    tags_prev = np.concatenate(
        [np.full((B, 1), START, np.int64), tags[:, :-1]], axis=1)
    pairval = trans[tags, tags_prev]                      # [B, S]
    smask = np.arange(S)[None, :] < lengths[:, None]
    trans_score = np.where(smask, pairval, 0.0).sum(axis=1)
    emit_vals = np.take_along_axis(feats, tags[:, :, None], axis=2)[:, :, 0]
    emit_score = np.where(smask, emit_vals, 0.0).sum(axis=1)
    return trans_score + emit_score


def kernel(feats, tags, lengths, transitions):
    global _compiled
    from concourse.bass_utils import run_bass_kernel_spmd
    import waitfix_embedded  # noqa: F401  (installs on import)

    if _compiled is None:
        _compiled = _build_bass()
    nc = _compiled
    in_maps = _host_inputs(feats, tags, lengths, transitions)
    res = run_bass_kernel_spmd(nc, in_maps, core_ids=list(range(NCORES)))

    lengths64 = np.asarray(lengths).astype(np.int64)
    gold = _gold(feats, tags, lengths, transitions)
    fwd = np.empty(B, np.float64)
    for core, r in enumerate(res.results):
        q = r["q_out"].astype(np.float64)                 # [G, 128]
        sl = slice(core * BC, (core + 1) * BC)
        fwd[sl] = np.log(q).reshape(BC) + C0 * lengths64[sl]
    return np.float32(np.mean(fwd - gold))


# ---- embedded waitfix module (kernel.py must be self-contained) ----
import types as _types  # noqa: E402

_wf_src = '''
import json

MAX_WAITS = 1

def dedupe_ldweights(bir):
    """Drop Ldweights that reload the weights already resident in the PE
    array (same stationary AP as the previous load, no intervening
    self-loading matmul). Their sync_info is merged into the next kept
    instruction so no dependency edges are lost."""
    def sig_of(inst):
        return json.dumps(
            [inst.get("ins"), inst.get("perf_mode"), inst.get("is_transpose"),
             inst.get("tile_position")], sort_keys=True)

    n_drop = 0
    for fn in bir["functions"]:
        for blk in fn["blocks"]:
            out = []
            last_sig = None
            pend_w, pend_u = [], []
            for inst in blk["instructions"]:
                op = inst.get("opcode")
                if op == "Ldweights":
                    sig = sig_of(inst)
                    if sig == last_sig:
                        si = inst.get("sync_info") or {}
                        pend_w += si.get("on_wait") or []
                        pend_u += si.get("on_update") or []
                        n_drop += 1
                        continue
                    last_sig = sig
                elif op in ("Matmult", "MatmultMx"):
                    if inst.get("is_transpose") or inst.get("ldweights"):
                        last_sig = None
                if pend_w or pend_u:
                    si = inst.get("sync_info")
                    if si is None:
                        si = {"on_update": [], "on_wait": []}
                        inst["sync_info"] = si
                    si["on_wait"] = (si.get("on_wait") or []) + pend_w
                    si["on_update"] = (si.get("on_update") or []) + pend_u
                    pend_w, pend_u = [], []
                out.append(inst)
            assert not (pend_w or pend_u), "dangling ldweights sync"
            blk["instructions"] = out
    return bir

def drop_satisfied_waits(bir):
    """Drop waits that are provably satisfied at issue: a wait on semaphore X
    by an instruction on engine E, where earlier instructions on E in the
    same block have already pushed X past the wait value (in-order queue).
    Semaphores touched by any non-inc update mode are left alone."""
    n_drop = 0
    for fn in bir["functions"]:
        for blk in fn["blocks"]:
            upd = {}      # (engine, sem_id) -> cumulative inc by that engine
            tainted = set()
            for inst in blk["instructions"]:
                e = inst.get("engine")
                si = inst.get("sync_info")
                if si and si.get("on_wait"):
                    keep = []
                    for w in si["on_wait"]:
                        if (w.get("sync_type") == "semaphore"
                                and w.get("wait_mode") == "sem-ge-imm"
                                and w["id"] not in tainted
                                and upd.get((e, w["id"]), 0) >= w["wait_value"]):
                            n_drop += 1
                            continue
                        keep.append(w)
                    si["on_wait"] = keep
                if inst.get("is_reset_sema"):
                    for u in (si or {}).get("on_update") or []:
                        tainted.add(u.get("id"))
                for u in (si or {}).get("on_update") or []:
                    if u.get("sync_type") != "semaphore":
                        continue
                    if u.get("update_mode") == "sem-inc":
                        k = (e, u["id"])
                        upd[k] = upd.get(k, 0) + u.get("update_value", 1)
                    else:
                        tainted.add(u.get("id"))
    return bir

def split_sync_waits(bir_bytes, max_waits=MAX_WAITS):
    bir = drop_satisfied_waits(dedupe_ldweights(json.loads(bir_bytes)))
    n_split = 0
    for fn in bir["functions"]:
        for blk in fn["blocks"]:
            out = []
            for inst in blk["instructions"]:
                si = inst.get("sync_info")
                waits = (si or {}).get("on_wait") or []
                if len(waits) > max_waits:
                    k = 0
                    while len(waits) > max_waits:
                        chunk, waits = waits[:max_waits], waits[max_waits:]
                        out.append({
                            "debug": inst.get("debug", 0),
                            "engine": inst["engine"],
                            "ins": [], "is_reset_sema": False,
                            "name": inst["name"] + "-wsplit%d" % k,
                            "opcode": "NoOp", "outs": [],
                            "sync_info": {"on_update": [], "on_wait": chunk},
                        })
                        k += 1
                    si["on_wait"] = waits
                    n_split += 1
                out.append(inst)
            blk["instructions"] = out
    return json.dumps(bir).encode()

def install():
    import concourse.bass2jax as bass2jax
    if getattr(bass2jax, "_waitfix_installed", False):
        return
    orig = bass2jax.compile_bir_kernel
    def patched(bir_json, tmpdir, neff_name="file.neff"):
        return orig(split_sync_waits(bir_json), tmpdir, neff_name)
    bass2jax.compile_bir_kernel = patched
    bass2jax._waitfix_installed = True

install()
'''
if "waitfix_embedded" not in sys.modules:
    _mod = _types.ModuleType("waitfix_embedded")
    exec(_wf_src, _mod.__dict__)
    sys.modules["waitfix_embedded"] = _mod


if __name__ == "__main__":
    import refcache
    inputs, exp = refcache.load()
    out = kernel(**inputs)
    rel = abs(float(out) - float(exp)) / max(abs(float(exp)), 1e-9)
    print("kernel:", out, "expected:", exp, "rel err:", rel)
